# revision 1
# baseline (speedup 1.0000x reference)
"""Trainium2 Bass kernel for pre-norm multi-head self-attention.

Reference computation (fp32, jax):
  xn = LayerNorm(x) * g + b
  qkv = xn @ W_qkv + b_qkv ; q,k,v = split(qkv); q *= d^-0.5
  out = softmax(q k^T) v          (12 heads, d=64)
  y = out @ W_out + b_out

Sharding: 8 cores = 4 batches x 2 head-groups (6 heads each).  Each core
computes its batch's LayerNorm + its 6 heads' attention and a *partial*
output projection; the host sums the two partials per batch and adds b_out.

Per-core design:
  - Two hard floors: ScalarE exp of the 6x2048x2048 scores (~200us busy)
    and the PE matmul stream (~226us busy at full clock).  The schedule
    keeps the PE stream dependency-free so it runs at full speed and
    paces the window, with ACT exp trailing at ~97% duty.
    (Counterintuitively, S^T in fp8 DoubleRow would SLOW the kernel: PE
    would drop below the ACT pace, idle every slab group, and the
    p-state ramp would knock it to half clock.  fp8 q/k/attn-weights
    also breach the 2e-2 error gate per host-side simulation.)
  - x is shipped bf16 from the host (halves the input DMA traffic).
  - LN via DVE bn_stats/bn_aggr; rsig via DVE reciprocal + ACT sqrt; the
    affine on ACT (Identity with per-partition scale+bias APs); x loads
    staggered on two DMA queues; transposes deferred two blocks so their
    waits never head-of-line-block an issue queue.
  - k/q for pair 0 and 12 of 16 v blocks are produced inside the LN
    window (PE is otherwise idle there).
  - S^T per head-pair via row-packed K=64 bf16 matmuls (tile_position
    (0,0)/(64,0)) into 2-bank PSUM slabs [128, 1024]; exp on ScalarE ->
    bf16 pa tiles; AV accumulates o[65, W] (+colsum row via the ones
    column in v) over j in PSUM.
  - Per slab group: slab fills emitted first (they pace ACT), AV runs 2
    groups deferred (its exp long done -> PE never stalls), and a
    cycle-budget pacer drips the remaining qk/v/proj work into the
    stream; ensure_v/ensure_qk force-drain whatever an AV or fill is
    about to depend on (pacing alone cannot order production before
    use).  The final pair-2 chunk is split in half so the un-overlapped
    normalize/projection drain after the last exp is shorter.
  - normalize: DVE reciprocal -> gpsimd partition_broadcast -> DVE mult
    (no PE, no PSUM); projection lhsT=OT chunk, DVE copies, DMA out.
  - Known HW pitfall: a K=1 PE bias-seed matmul + ACT Identity-copy
    variant of q/k/v production corrupted results on real hardware
    (invisible to the interpreter); bias-adds stay on DVE.
"""

import sys

sys.path.insert(0, "/opt/trn_rl_repo")

import numpy as np
import ml_dtypes

import concourse.bass as bass
import concourse.bacc as bacc
import concourse.mybir as mybir
import concourse.tile as tile
from concourse.bass_utils import run_bass_kernel_spmd

F32 = mybir.dt.float32
BF16 = mybir.dt.bfloat16
AX = mybir.AxisListType
ALU = mybir.AluOpType
ACTF = mybir.ActivationFunctionType

B, N, DIM = 4, 2048, 768
HEADS, DH = 12, 64
HPC = 6          # heads per core
GQ = HPC * DH    # 384: per-core q/k/v width
PB = 128         # partition block
IC = 512         # i-chunk (PSUM bank width in fp32)
NFC = DIM // PB  # 6 feature chunks
EPS = 1e-5
PA_BUFS = 8      # bf16 [128,1024] staging tiles for exp(S^T)
HEAD_V = 12      # v blocks produced during the LN window
ATTN_IN_LN = False  # stream first attention groups inside the LN loop --
                    # gains ~117ns in-model but races nondeterministically
                    # on real HW (one run passed, the next failed at 0.139)
LNEXP_RSIG = False  # rsig via exp(-0.5*ln(v)) on ACT vs DVE-recip + ACT-sqrt
X_ALL_POOL = False  # issue every x load on the gpsimd queue


def build_nc(n=N):
    nb = n // PB
    nic = n // IC
    nc = bacc.Bacc("TRN2", target_bir_lowering=False, debug=False)

    x_d = nc.dram_tensor("x", [n, DIM], BF16, kind="ExternalInput")
    wqk_d = nc.dram_tensor("wqk", [DIM, 2 * GQ], BF16, kind="ExternalInput")
    wv_d = nc.dram_tensor("wv", [DIM, GQ], BF16, kind="ExternalInput")
    bqk_d = nc.dram_tensor("bqk", [1, 2 * GQ], BF16, kind="ExternalInput")
    bv_d = nc.dram_tensor("bv", [1, GQ], BF16, kind="ExternalInput")
    bqkp_d = nc.dram_tensor("bqkp", [PB, 6], F32, kind="ExternalInput")
    wo_d = nc.dram_tensor("wo", [GQ, DIM], BF16, kind="ExternalInput")
    out_d = nc.dram_tensor("out", [n, DIM], F32, kind="ExternalOutput")

    with tile.TileContext(nc) as tc:
        _body(nc, tc, n, nb, nic, x_d, wqk_d, wv_d, bqk_d, bv_d,
              bqkp_d, wo_d, out_d)
    nc.compile()
    return nc


def _body(nc, tc, n, nb, nic, x_d, wqk_d, wv_d, bqk_d, bv_d,
          bqkp_d, wo_d, out_d):
    with (
        tc.tile_pool(name="const", bufs=1) as cpool,
        tc.tile_pool(name="persist", bufs=1) as perm,
        tc.tile_pool(name="ln", bufs=4) as lnp,
        tc.tile_pool(name="pa", bufs=PA_BUFS) as pap,
        tc.tile_pool(name="nrm", bufs=4) as nrm,
        tc.tile_pool(name="ps", bufs=2, space="PSUM") as pp,
    ):
        # ---- constants / weights ----
        zbias = cpool.tile([PB, 1], F32, tag="zb")
        nc.vector.memset(zbias[:], 0.0)
        ones_row = cpool.tile([1, IC], BF16, tag="ones_row")
        nc.vector.memset(ones_row[:], 1.0)

        # bias ROWS (bf16); bv is broadcast across partitions on-device
        bqk_sb = cpool.tile([1, 2 * GQ], BF16, tag="bqk")
        nc.sync.dma_start(bqk_sb[:], bqk_d[:, :])
        bv_sb = cpool.tile([1, GQ], BF16, tag="bv")
        nc.sync.dma_start(bv_sb[:], bv_d[:, :])
        # per-partition bias layouts for the window-phase DVE bias-adds.
        # bvp is the row broadcast across partitions -- built on-device
        # (emitted after the x loads so it doesn't delay the Pool queue).
        bqkp_sb = cpool.tile([PB, 6], F32, tag="bqkp")
        nc.sync.dma_start(bqkp_sb[:], bqkp_d[:, :])
        bvp_sb = cpool.tile([PB, GQ], BF16, tag="bvp")

        wqk_sb = []
        wv_sb = []
        wo_sb = []
        for kc in range(NFC):
            t = cpool.tile([PB, 2 * GQ], BF16, tag=f"wqk{kc}")
            nc.sync.dma_start(t[:], wqk_d[kc * PB:(kc + 1) * PB, :])
            wqk_sb.append(t)

        def load_wv():
            for kc in range(NFC):
                t = cpool.tile([PB, GQ], BF16, tag=f"wv{kc}")
                nc.sync.dma_start(t[:], wv_d[kc * PB:(kc + 1) * PB, :])
                wv_sb.append(t)

        def load_wo():
            for p in range(3):
                t = cpool.tile([PB, DIM], BF16, tag=f"wo{p}")
                nc.sync.dma_start(t[:], wo_d[p * PB:(p + 1) * PB, :])
                wo_sb.append(t)

        # ---- persistent activations ----
        xnT_all = perm.tile([PB, NFC * n], BF16, tag="xnT_all", name="xnT_all")
        xnT = [xnT_all[:, kc * n:(kc + 1) * n] for kc in range(NFC)]
        # qkT[0..2] = q pairs (head 2p rows 0:64, head 2p+1 rows 64:128),
        # qkT[3..5] = k pairs
        qkT = [perm.tile([PB, n], BF16, tag=f"qkT{mc}", name=f"qkT{mc}") for mc in range(6)]
        v_sb = [perm.tile([PB, HPC * 65], BF16, tag=f"v{jb}", name=f"v{jb}")
                for jb in range(nb)]
        OT = [perm.tile([PB, n], BF16, tag=f"OT{p}", name=f"OT{p}") for p in range(3)]

        # ---- LayerNorm.  All x loads are issued up front on two queues so
        # no load issue ever queues behind a waiting compute instruction. ----
        xts = []
        for ib in range(nb):
            xt = lnp.tile([PB, DIM], BF16, tag="x", bufs=nb, name=f"xt{ib}")
            xts.append(xt)

        def load_x(ib):
            eng = nc.gpsimd if (ib % 2 == 0 or X_ALL_POOL) else nc.scalar
            eng.dma_start(xts[ib][:], x_d[ib * PB:(ib + 1) * PB, :])

        # first four loads up front; the rest staggered into the LN loop so
        # no compute queue is head-blocked by a burst of DMA issues
        for ib in range(4):
            load_x(ib)
        nc.gpsimd.partition_broadcast(bvp_sb[:], bv_sb[:])

        # ones columns of [v_h | 1] tiles (tiny DVE memsets, before any AV)
        for jb in range(nb):
            col = v_sb[jb][:].rearrange("p (h c) -> p h c", c=65)[:, :, 64:65]
            nc.vector.memset(col, 1.0)

        def ln_stage1(ib):
            xt = xts[ib]
            st = lnp.tile([PB, 12], F32, tag="st", name=f"st{ib}")
            nc.vector.bn_stats(st[:, 0:6], xt[:, 0:384])
            nc.vector.bn_stats(st[:, 6:12], xt[:, 384:768])
            mv = lnp.tile([PB, 2], F32, tag="mv", name=f"mv{ib}")
            nc.vector.bn_aggr(mv[:], st[:])
            veps = lnp.tile([PB, 1], F32, tag="veps", name=f"veps{ib}")
            nc.vector.tensor_scalar_add(veps[:], mv[:, 1:2], EPS)
            if LNEXP_RSIG:
                # rsig = exp(-0.5 * ln(var+eps)); Ln and Exp share one ACT
                # table with Identity/Copy, so the whole kernel needs a
                # single activation-table load (Sqrt would thrash against
                # the attention exps once the streams interleave).
                lnv = lnp.tile([PB, 1], F32, tag="lnv", name=f"lnv{ib}")
                nc.scalar.activation(lnv[:], veps[:], ACTF.Ln)
                rs = lnp.tile([PB, 1], F32, tag="rs", name=f"rs{ib}")
                nc.scalar.activation(rs[:], lnv[:], ACTF.Exp, scale=-0.5)
            else:
                rv = lnp.tile([PB, 1], F32, tag="rv", name=f"rv{ib}")
                nc.vector.reciprocal(rv[:], veps[:])
                rs = lnp.tile([PB, 1], F32, tag="rs", name=f"rs{ib}")
                nc.scalar.sqrt(rs[:], rv[:])
            return xt, mv, rs

        def ln_stage2(ib, xt, mv, rs):
            nmr = lnp.tile([PB, 1], F32, tag="nmr", name=f"nmr{ib}")
            nc.vector.tensor_scalar(
                out=nmr[:], in0=mv[:, 0:1], scalar1=rs[:], scalar2=-1.0,
                op0=ALU.mult, op1=ALU.mult,
            )
            xnt = lnp.tile([PB, DIM], BF16, tag="xn", bufs=6, name=f"xn{ib}")
            nc.scalar.activation(
                xnt[:], xt[:], ACTF.Identity, bias=nmr[:], scale=rs[:],
            )
            return xnt

        def ln_stage3(ib, xnt):
            # transpose deferred 2 blocks: its wait never head-of-line
            # blocks the issuing queue
            tout = xnT_all[:].rearrange("p (k i) -> p k i", i=n)[:, :, ib * PB:(ib + 1) * PB]
            eng = nc.scalar if ib % 2 == 0 else nc.sync
            eng.dma_start_transpose(tout, xnt[:])

        # ---- q/k production (bf16; fp8 inputs would breach the err gate).
        # In the LN head the bias is seeded into PSUM via a K=1 outer-product
        # matmul and the PSUM->SBUF copy alternates ACT/DVE (keeps head DVE
        # light); in the attention window the plain DVE bias-add is used
        # instead (PE paces the window, DVE has slack there).
        def make_qk(mc, ics, eng=None):
            eng = eng or nc.vector
            for ic in ics:
                ps = pp.tile([PB, IC], F32, tag="acc", name=f"qkps{mc}_{ic}", bufs=4)
                for kc in range(NFC):
                    nc.tensor.matmul(
                        ps[:],
                        wqk_sb[kc][:, mc * PB:(mc + 1) * PB],
                        xnT[kc][:, ic * IC:(ic + 1) * IC],
                        start=(kc == 0), stop=(kc == NFC - 1),
                    )
                eng.tensor_scalar_add(
                    qkT[mc][:, ic * IC:(ic + 1) * IC], ps[:], bqkp_sb[:, mc:mc + 1],
                )

        # ---- v (natural layout, strided into [v_h | 1] tiles) ----
        def make_v(jbs, eng=None):
            eng = eng or nc.vector
            for jb in jbs:
                ps = pp.tile([PB, GQ], F32, tag="acc", name=f"vps{jb}", bufs=4)
                for kc in range(NFC):
                    nc.tensor.matmul(
                        ps[:],
                        xnT[kc][:, jb * PB:(jb + 1) * PB],
                        wv_sb[kc][:],
                        start=(kc == 0), stop=(kc == NFC - 1),
                    )
                dst = v_sb[jb][:, 0:HPC * 65].rearrange("p (h c) -> p h c", c=65)[:, :, 0:64]
                eng.tensor_tensor(
                    out=dst,
                    in0=ps[:].rearrange("p (h c) -> p h c", c=64),
                    in1=bvp_sb[:].rearrange("p (h c) -> p h c", c=64),
                    op=ALU.add,
                )

        def proj_pieces(ib):
            ibsl = slice(ib * PB, (ib + 1) * PB)
            cell = {}

            def piece(half):
                f = pp.tile([PB, 384], F32, tag="acc", name=f"f{half}", bufs=4)
                for pj in range(3):
                    nc.tensor.matmul(
                        f[:], OT[pj][:, ibsl], wo_sb[pj][:, half * 384:(half + 1) * 384],
                        start=(pj == 0), stop=(pj == 2),
                    )
                if half == 0:
                    cell["ot"] = nrm.tile([PB, DIM], F32, tag="out", name="ot")
                ot = cell["ot"]
                nc.vector.tensor_copy(ot[:, half * 384:(half + 1) * 384], f[:])
                if half == 1:
                    nc.sync.dma_start(out_d[ibsl, :], ot[:])
            return [(lambda h=h: piece(h), 1152) for h in (0, 1)]

        def normalize(pn, st, W, o_An, o_Bn):
            isn = slice(st, st + W)
            rcs = []
            for o_X in (o_An, o_Bn):
                rc = nrm.tile([1, W], F32, tag="recip", name="rc")
                nc.vector.reciprocal(rc[:], o_X[64:65, :])
                rcs.append(rc)
            rbs = []
            for rc in rcs:
                rb = nrm.tile([64, W], F32, tag="rb", name="rb")
                nc.gpsimd.partition_broadcast(rb[:], rc[:])
                rbs.append(rb)
            for (o_X, half), rb in zip(((o_An, 0), (o_Bn, 1)), rbs):
                nc.vector.tensor_tensor(
                    out=OT[pn][half * 64:(half + 1) * 64, isn],
                    in0=o_X[0:64, :], in1=rb[:], op=ALU.mult,
                )
            if pn == 2:
                out = []
                for ib in range(st // PB, (st + W) // PB):
                    out.extend(proj_pieces(ib))
                return out
            return []

        # work queues: (closure, pe_cycles); a budget pacer drips these into
        # the PE stream between slab fills, and ensure_v/ensure_qk force-pop
        # whatever an AV or fill is about to depend on (the budget alone
        # cannot guarantee production precedes use).
        # (filled only after the LN loop; the in-LN generator steps see
        # empty queues and just stream slab groups)
        work_v = []   # (fn, cost, jb), ascending jb
        work_qk = {1: [], 2: []}

        def ensure_v(jb):
            while work_v and work_v[0][2] <= jb:
                fn, cost, _ = work_v.pop(0)
                state["budget"] -= cost
                fn()

        def ensure_qk(p):
            for fn, cost in work_qk.get(p, ()):
                state["budget"] -= cost
                fn()
            work_qk[p] = []

        def qk_pieces(mc, ic):
            cell = {}

            def piece(kc2):
                if kc2 == 0:
                    cell["ps"] = pp.tile(
                        [PB, IC], F32, tag="acc", name=f"qkps{mc}_{ic}", bufs=4,
                    )
                ps = cell["ps"]
                for kc in (kc2, kc2 + 1):
                    nc.tensor.matmul(
                        ps[:],
                        wqk_sb[kc][:, mc * PB:(mc + 1) * PB],
                        xnT[kc][:, ic * IC:(ic + 1) * IC],
                        start=(kc == 0), stop=(kc == NFC - 1),
                    )
                if kc2 + 2 >= NFC:
                    nc.vector.tensor_scalar_add(
                        qkT[mc][:, ic * IC:(ic + 1) * IC], ps[:],
                        bqkp_sb[:, mc:mc + 1],
                    )
            return [(lambda kc2=kc2: piece(kc2), 1024) for kc2 in range(0, NFC, 2)]

        def v_pieces(jb):
            cell = {}

            def piece(kc3):
                if kc3 == 0:
                    cell["ps"] = pp.tile(
                        [PB, GQ], F32, tag="acc", name=f"vps{jb}", bufs=4,
                    )
                ps = cell["ps"]
                for kc in (kc3, kc3 + 1, kc3 + 2):
                    nc.tensor.matmul(
                        ps[:],
                        xnT[kc][:, jb * PB:(jb + 1) * PB],
                        wv_sb[kc][:],
                        start=(kc == 0), stop=(kc == NFC - 1),
                    )
                if kc3 + 3 >= NFC:
                    dst = v_sb[jb][:, 0:HPC * 65].rearrange("p (h c) -> p h c", c=65)[:, :, 0:64]
                    nc.vector.tensor_tensor(
                        out=dst,
                        in0=ps[:].rearrange("p (h c) -> p h c", c=64),
                        in1=bvp_sb[:].rearrange("p (h c) -> p h c", c=64),
                        op=ALU.add,
                    )
            return [(lambda kc3=kc3: piece(kc3), 1152) for kc3 in range(0, NFC, 3)]

        # ---- attention (generator: one slab group per step, so the first
        # chunk's groups can interleave with the LN-loop emission) ----
        state = {"pending": None, "budget": 0.0}
        proj_q = []

        def emit_chunk(p, st, W, defer=2):
            qt, kt = qkT[p], qkT[3 + p]
            isl = slice(st, st + W)
            if state["pending"] is not None:
                proj_q.extend(normalize(*state["pending"]))
                state["pending"] = None
            o_A = pp.tile([65, W], F32, tag="acc", bufs=4, name="o_A")
            o_B = pp.tile([65, W], F32, tag="acc", bufs=4, name="o_B")
            o_h = (o_A, o_B)
            pa_hist = {}

            def fill_exp(h, g):
                slab = pp.tile([PB, 2 * W], F32, tag="slab", name=f"slab{h}")
                for u in range(2):
                    jb = 2 * g + u
                    jsl = slice(jb * PB, (jb + 1) * PB)
                    nc.tensor.matmul(
                        slab[:, u * W:(u + 1) * W],
                        kt[h * 64:(h + 1) * 64, jsl],
                        qt[h * 64:(h + 1) * 64, isl],
                        tile_position=(h * 64, 0),
                    )
                ta = pap.tile([PB, 2 * W], BF16, tag="pa", name="pa_t")
                nc.scalar.activation(ta[:], slab[:], ACTF.Exp, bias=zbias[:])
                pa_hist[(h, g)] = ta

            def outp_mm(h, g):
                ensure_v(2 * g + 1)
                pa = pa_hist.pop((h, g))
                for u in range(2):
                    jb = 2 * g + u
                    nc.tensor.matmul(
                        o_h[h][:], v_sb[jb][:, (2 * p + h) * 65:(2 * p + h + 1) * 65],
                        pa[:, u * W:(u + 1) * W],
                        start=(jb == 0), stop=(jb == nb - 1),
                    )

            ng = nb // 2
            for g in range(ng):
                fill_exp(0, g)
                fill_exp(1, g)
                if g >= defer:
                    outp_mm(0, g - defer)
                    outp_mm(1, g - defer)
                state["budget"] = min(state["budget"] + 1000.0 * W / IC, 6000.0)
                while True:
                    if proj_q and state["budget"] >= proj_q[0][1]:
                        fn, cost = proj_q.pop(0)
                        state["budget"] -= cost
                        fn()
                    elif work_v and state["budget"] >= work_v[0][1]:
                        fn, cost, _ = work_v.pop(0)
                        state["budget"] -= cost
                        fn()
                    elif work_qk[1] and state["budget"] >= work_qk[1][0][1]:
                        fn, cost = work_qk[1].pop(0)
                        state["budget"] -= cost
                        fn()
                    elif work_qk[2] and state["budget"] >= work_qk[2][0][1]:
                        fn, cost = work_qk[2].pop(0)
                        state["budget"] -= cost
                        fn()
                    else:
                        break
                yield
            for g in range(ng - defer, ng):
                outp_mm(0, g)
                outp_mm(1, g)
            state["pending"] = (p, st, W, o_A, o_B)

        def all_chunks():
            for p in range(3):
                ensure_qk(p)
                if p < 2:
                    chunks = [(ic * IC, IC, 2) for ic in range(nic)]
                else:
                    # halve the final chunk so the un-overlapped normalize/
                    # projection drain after the last exp is half as long
                    chunks = [(ic * IC, IC, 2) for ic in range(nic - 1)]
                    h = IC // 2
                    chunks += [((nic - 1) * IC, h, 2), ((nic - 1) * IC + h, h, 1)]
                for st, W, defer in chunks:
                    yield from emit_chunk(p, st, W, defer)

        # ---- emission: LN pipelined with k0/q0/v head work (PE is idle
        # during LN otherwise; this trims the attention-phase work so the
        # PE stream there stays just above the ACT exp pace) ----
        def head_work(w):
            # plain DVE bias-adds: a K=1 PE bias-seed + ACT-copy variant
            # corrupted results on real HW, so production stays on DVE
            make_qk(3, [w])
            make_qk(0, [w])
            lo = w * (HEAD_V // nic + 1)
            hi = min(HEAD_V, (w + 1) * (HEAD_V // nic + 1))
            make_v(range(min(lo, HEAD_V), hi))

        attn_gen = all_chunks()

        stage = {}
        for ib in range(nb):
            if ib + 4 < nb:
                load_x(ib + 4)
            stage[ib] = (ln_stage1(ib),)
            if ib - 1 in stage and len(stage[ib - 1]) == 1:
                stage[ib - 1] += (ln_stage2(ib - 1, *stage[ib - 1][0]),)
            if ib - 2 in stage:
                ln_stage3(ib - 2, stage.pop(ib - 2)[1])
            if ib == 2:
                load_wv()
            if ib % 4 == 1 and ib > 4:
                head_work((ib - 1) // 4 - 1)
            if ATTN_IN_LN and ib == 15:
                # overlap the first two attention groups with the LN tail
                # (k chunk 0 and q chunk 0 exist by then; earlier advances
                # displace LN affines one-for-one and measure worse)
                next(attn_gen)
                next(attn_gen)
        stage[nb - 1] += (ln_stage2(nb - 1, *stage[nb - 1][0]),)
        ln_stage3(nb - 2, stage.pop(nb - 2)[1])
        ln_stage3(nb - 1, stage.pop(nb - 1)[1])
        load_wo()
        head_work(nic - 1)

        for jb in range(HEAD_V, nb):
            for fn, cost in v_pieces(jb):
                work_v.append((fn, cost, jb))
        for p in (1, 2):
            for ic in range(nic):
                work_qk[p].extend(qk_pieces(3 + p, ic))
            for ic in range(nic):
                work_qk[p].extend(qk_pieces(p, ic))

        for _ in attn_gen:
            pass
        ensure_v(nb)
        ensure_qk(1)
        ensure_qk(2)
        proj_q.extend(normalize(*state["pending"]))
        while proj_q:
            proj_q.pop(0)[0]()


# ------------------------------------------------------------------ host side

_NC_CACHE = {}


def _get_nc(n=N):
    if n not in _NC_CACHE:
        _NC_CACHE[n] = build_nc(n)
    return _NC_CACHE[n]


def make_in_maps(x, ln_g, ln_b, W_qkv, b_qkv, W_out):
    """Fold LN affine + q-scale into weights; build the 8 per-core input maps."""
    bf16 = ml_dtypes.bfloat16
    W_eff = (np.asarray(ln_g)[:, None] * np.asarray(W_qkv)).astype(np.float32)
    b_eff = (np.asarray(ln_b) @ np.asarray(W_qkv) + np.asarray(b_qkv)).astype(np.float32)
    scale = 1.0 / np.sqrt(DH)
    in_maps = []
    for b in range(B):
        for g in range(2):
            qs = slice(g * GQ, (g + 1) * GQ)
            ks = slice(768 + g * GQ, 768 + (g + 1) * GQ)
            vs = slice(1536 + g * GQ, 1536 + (g + 1) * GQ)
            wqk = np.concatenate(
                [W_eff[:, qs] * scale, W_eff[:, ks]], axis=1
            ).astype(bf16)
            wv = W_eff[:, vs].astype(bf16)
            bqk_full = np.concatenate([b_eff[qs] * scale, b_eff[ks]])
            bqk = bqk_full.reshape(1, 2 * GQ).astype(bf16)
            bv = b_eff[vs].reshape(1, GQ).astype(bf16)
            bqkp = np.ascontiguousarray(
                bqk_full.reshape(6, PB).T
            ).astype(np.float32)
            wo = np.asarray(W_out)[g * GQ:(g + 1) * GQ, :].astype(bf16)
            in_maps.append({
                "x": np.ascontiguousarray(np.asarray(x)[b]).astype(bf16),
                "wqk": np.ascontiguousarray(wqk),
                "wv": np.ascontiguousarray(wv),
                "bqk": bqk,
                "bv": bv,
                "bqkp": bqkp,
                "wo": np.ascontiguousarray(wo),
            })
    return in_maps


def _run(inputs, trace=False):
    in_maps = make_in_maps(
        inputs["x"], inputs["ln_g"], inputs["ln_b"],
        inputs["W_qkv"], inputs["b_qkv"], inputs["W_out"],
    )
    nc = _get_nc(N)
    res = run_bass_kernel_spmd(nc, in_maps, core_ids=list(range(8)), trace=trace)
    out = np.empty((B, N, DIM), np.float32)
    for b in range(B):
        out[b] = res.results[2 * b]["out"] + res.results[2 * b + 1]["out"]
    out += np.asarray(inputs["b_out"], dtype=np.float32)[None, None, :]
    return out, res


def kernel(**inputs):
    out, _ = _run(inputs, trace=False)
    return out


def run_traced(**inputs):
    return _run(inputs, trace=True)

